# revision 1
# baseline (speedup 1.0000x reference)
"""Trainium2 Bass kernel for nn_MultiHeadAttention_79130477461654.

The reference einsum "nhqk,nhvd->nhqd" contracts k and v independently, so
out = (sum_k softmax(energy))*(sum_s v) = broadcast(sum_s v) since softmax
rows sum to 1.  With v = split_heads(x @ Wv) and the reference's direct
(n,h,q,d)->(n,s,e) reshape, the full output reduces to

    xs[n]    = sum_s x[n,s,:]                       (1024,)
    Sfull[n] = xs[n] @ Wv                           (1024,)
    WoSum    = sum_m Wo[64m+d, :]  (d=0..63)        (64, 1024)
    T[n,h,:] = Sfull[n][64h:64h+64] @ WoSum + bo    (16, 1024)
    out[n, 64h+r, :] = T[n,h,:]   for r in 0..63

numerically within ~1e-4 of the reference (softmax-row-sum rounding +
fp32r matmul rounding).  Sharding: data parallel over batch N=8, one
batch per core; Wv/Wo replicated.  All arithmetic on-device; reductions
run on the PE array chasing the DMA arrivals.
"""

import numpy as np

N, S, E, H, D = 8, 1024, 1024, 16, 64
NCORES = 8
P = 128  # partitions


def build_nc():
    import concourse.bacc as bacc
    import concourse.mybir as mybir
    from concourse.tile import TileContext

    F32 = mybir.dt.float32
    F32R = mybir.dt.float32r
    nc = bacc.Bacc("TRN2", target_bir_lowering=False, debug=False)

    xd = nc.declare_dram_parameter("x", [S, E], F32, isOutput=False)
    wvd = nc.declare_dram_parameter("Wv", [E, E], F32, isOutput=False)
    wod = nc.declare_dram_parameter("Wo", [E, E], F32, isOutput=False)
    bod = nc.declare_dram_parameter("bo128", [P, E], F32, isOutput=False)
    onesd = nc.declare_dram_parameter("ones128", [P, 1], F32, isOutput=False)
    dbld = nc.declare_dram_parameter("dblI", [P, D], F32, isOutput=False)
    outd = nc.declare_dram_parameter("out", [S, E], F32, isOutput=True)

    # two HWDGE queues: SP (sync) and ACT (scalar)
    dmae = [nc.sync, nc.scalar]

    # DRAM-side views pairing two 128-row chunks per 1 MB transfer:
    # paired(src, i)[p, c, :] = src[(2i + c)*128 + p, :]
    def paired(dram, i):
        return dram.rearrange("(i c p) e -> i p c e", p=P, c=2)[i]

    with TileContext(nc) as tc:
        with (
            tc.tile_pool(name="xin", bufs=4) as xp,
            tc.tile_pool(name="wv", bufs=4) as wvp,
            tc.tile_pool(name="wo", bufs=4) as wop,
            tc.tile_pool(name="small", bufs=1) as sp,
            tc.tile_pool(name="outsb", bufs=2) as op,
            tc.tile_pool(name="psA", bufs=1, space="PSUM") as psA,
            tc.tile_pool(name="psS", bufs=1, space="PSUM") as psS,
            tc.tile_pool(name="psF", bufs=1, space="PSUM") as psF,
            tc.tile_pool(name="psO", bufs=3, space="PSUM") as psO,
        ):
            ones_sb = sp.tile([P, 1], F32)
            dmae[0].dma_start(out=ones_sb[:], in_=onesd[:])
            dbl_sb = sp.tile([P, D], F32)
            dmae[1].dma_start(out=dbl_sb[:], in_=dbld[:])
            bo_sb = sp.tile([P, E], F32)
            dmae[1].dma_start(out=bo_sb[:], in_=bod[:])
            dbl_r = sp.tile([P, D], F32R)
            nc.vector.tensor_copy(dbl_r[:], dbl_sb[:])

            # ---- input DMAs: x, Wv, Wo as 1 MB paired transfers, 2 per queue
            #      x tiles reduce pairwise on DVE (chasing the DMAs):
            #      xacc2[p, cp*E + e] = sum_i x[(2i+cp)*128 + p, e]
            xacc2 = sp.tile([P, 2 * E], F32)
            wvt, wot = [], []
            for i in range(4):
                t = xp.tile([P, 2 * E], F32)
                dmae[i % 2].dma_start(
                    out=t[:].rearrange("p (c e) -> p c e", c=2), in_=paired(xd, i)
                )
                if i == 0:
                    nc.vector.tensor_copy(xacc2[:], t[:])
                else:
                    nc.vector.tensor_add(xacc2[:], xacc2[:], t[:])
            for i in range(4):
                t = wvp.tile([P, 2 * E], F32, tag="wvf")
                dmae[i % 2].dma_start(
                    out=t[:].rearrange("p (c e) -> p c e", c=2), in_=paired(wvd, i)
                )
                tr = wvp.tile([P, 2 * E], F32R, tag="wvr")
                nc.vector.tensor_copy(tr[:], t[:])
                wvt.append(tr)
            for i in range(4):
                t = wop.tile([P, 2 * E], F32, tag="wof")
                dmae[i % 2].dma_start(
                    out=t[:].rearrange("p (c e) -> p c e", c=2), in_=paired(wod, i)
                )
                tr = wop.tile([P, 2 * E], F32R, tag="wor")
                nc.vector.tensor_copy(tr[:], t[:])
                wot.append(tr)

            # ---- xsT[p, c] = xs[128c + p]: PE partition-reduction.
            #      Per-column groups are CONTIGUOUS (a start=True clears the
            #      whole PSUM bank's has_written, so groups sharing a bank
            #      must not interleave).
            ps_xsT = psA.tile([P, 8], F32, tag="psa")
            for c in range(8):
                for cp in range(2):
                    nc.tensor.matmul(
                        ps_xsT[:, c : c + 1],
                        xacc2[:, cp * E + c * P : cp * E + (c + 1) * P],
                        ones_sb[:],
                        start=(cp == 0),
                        stop=(cp == 1),
                    )
            xsT = sp.tile([P, 8], F32R)
            nc.vector.tensor_copy(xsT[:], ps_xsT[:])

            # ---- Sfull row (1, 1024) = xs @ Wv  (wide fp32r, chases Wv DMA)
            ps_S = psS.tile([1, E], F32, tag="pss")
            for c in range(8):
                base = (c % 2) * E
                for half in range(2):
                    sl = slice(half * 512, half * 512 + 512)
                    nc.tensor.matmul(
                        ps_S[0:1, sl],
                        xsT[:, c : c + 1],
                        wvt[c // 2][:, base + half * 512 : base + half * 512 + 512],
                        start=(c == 0),
                        stop=(c == 7),
                        skip_group_check=True,
                    )
            srow = sp.tile([1, E], F32)
            nc.vector.tensor_copy(srow[:], ps_S[:])

            # ---- sft[d, h] = Sfull[64h + d]  (N=1 fp32, base partition 0)
            ps_sft = psA.tile([D, H], F32, tag="psa")
            for h in range(H):
                nc.tensor.matmul(
                    ps_sft[:, h : h + 1],
                    srow[0:1, h * D : (h + 1) * D],
                    ones_sb[0:1, 0:1],
                    start=True,
                    stop=True,
                )
            sft = sp.tile([D, H], F32)
            nc.vector.tensor_copy(sft[:], ps_sft[:])

            # ---- rep[d, 64h + r] = sft[d, h]  (DVE free-dim broadcast, fp32r out)
            rep = sp.tile([D, H * D], F32R)
            nc.vector.tensor_copy(
                rep[:].rearrange("d (h r) -> d h r", r=D),
                sft[:, :, None].to_broadcast((D, H, D)),
            )

            # ---- WoSum[d, :] = sum_m Wo[64m + d, :]: PE fold with double
            #      identity, chasing the Wo DMAs (wide fp32r)
            ps_fold = psF.tile([D, E], F32, tag="psf")
            for i in range(4):
                for cp in range(2):
                    k = 2 * i + cp
                    for half in range(2):
                        sl = slice(half * 512, half * 512 + 512)
                        nc.tensor.matmul(
                            ps_fold[:, sl],
                            dbl_r[:],
                            wot[i][:, cp * E + half * 512 : cp * E + half * 512 + 512],
                            start=(k == 0),
                            stop=(k == 7),
                            skip_group_check=True,
                        )
            wosum = sp.tile([D, E], F32R)
            nc.vector.tensor_copy(wosum[:], ps_fold[:])

            # ---- fused T+broadcast, two 128-row blocks per 1 MB output DMA
            outr = outd.rearrange("(i c p) e -> i p c e", p=P, c=2)
            for i in range(4):
                ob = op.tile([P, 2 * E], F32)
                for c in range(2):
                    t = 2 * i + c
                    for half in range(2):
                        sl = slice(half * 512, half * 512 + 512)
                        po = psO.tile([P, 512], F32, tag="pso")
                        nc.tensor.matmul(
                            po[:],
                            rep[:, t * P : (t + 1) * P],
                            wosum[:, sl],
                            start=True,
                            stop=True,
                        )
                        # bias add fused with PSUM->SBUF move
                        nc.vector.tensor_add(
                            ob[:, c * E + half * 512 : c * E + half * 512 + 512],
                            po[:],
                            bo_sb[:, sl],
                        )
                dmae[i % 2].dma_start(
                    out=outr[i], in_=ob[:].rearrange("p (c e) -> p c e", c=2)
                )

    nc.compile()
    return nc


_NC_CACHE = None


def make_in_maps(x, Wv, Wo, bo):
    x = np.ascontiguousarray(np.asarray(x, dtype=np.float32))
    Wv = np.ascontiguousarray(np.asarray(Wv, dtype=np.float32))
    Wo = np.ascontiguousarray(np.asarray(Wo, dtype=np.float32))
    bo = np.ascontiguousarray(np.asarray(bo, dtype=np.float32))
    bo128 = np.tile(bo.reshape(1, E), (P, 1))
    ones128 = np.ones((P, 1), dtype=np.float32)
    dblI = np.zeros((P, D), dtype=np.float32)
    dblI[np.arange(P), np.arange(P) % D] = 1.0
    return [
        {
            "x": np.ascontiguousarray(x[j]),
            "Wv": Wv,
            "Wo": Wo,
            "bo128": bo128,
            "ones128": ones128,
            "dblI": dblI,
        }
        for j in range(NCORES)
    ]


def kernel(x, Wq=None, Wk=None, Wv=None, Wo=None, bo=None, **_unused):
    from concourse.bass_utils import run_bass_kernel_spmd

    global _NC_CACHE
    if _NC_CACHE is None:
        _NC_CACHE = build_nc()
    nc = _NC_CACHE

    in_maps = make_in_maps(x, Wv, Wo, bo)
    res = run_bass_kernel_spmd(nc, in_maps, core_ids=list(range(NCORES))).results
    return np.stack([res[j]["out"] for j in range(NCORES)], axis=0)

